# revision 5
# baseline (speedup 1.0000x reference)
"""Trainium2 Bass kernel for GraphTransformerNet (star-graph TransformerConv).

Shapes (hardcoded): B=1024 graphs, N=128 neighbors, D=256 in-dim,
H=4 heads x C=64 = F=256 out-dim. Data-parallel over 8 NeuronCores
(128 graphs/core).

v3 — fp8 DoubleRow pipeline:
  All big matmuls run in fp8e4m3 with MatmulPerfMode.DoubleRow, which
  packs the full D=256 contraction into one matmul at 0.5 cycles/row
  (4x the bf16 two-chunk stream). Accuracy is held by a compensated
  split: x ships as x8=fp8(x) plus r8=fp8(x-x8); weights are pre-scaled
  by 16 (their sigma is 1/16 -- unscaled fp8 lands subnormal) and split
  Ws8+Wsr8. skip = x8@Ws8 + x8@Wsr8 + r8@Ws8 accumulates in PSUM at
  16x scale; the host folds the 1/16 into its output pass. The e path
  (k/v edge contribution) is plain fp8 -- it only feeds the central row
  (1/129 of the output norm). Scores reuse the same stationaries with
  per-graph folded q columns (wkq8/weq8, 16x); Act's Exp applies the
  1/16 rescale for free. Aggregation keeps bf16 v/exp: v-slices are
  paired [128,128] stationaries (2 ldweights per graph, hidden under
  the produce stream); each 1-col matmul writes a full column of a
  4-region PSUM where half the rows are garbage, sliced out host-side.

  Per-graph PE stream: 5x 256-col DoubleRow mms + 2 tiny score mms +
  4 tiny agg mms ~= 400ns, under the DMA floor. Inputs ship as
  [128p, 2k, NG, x8|r8|e8] fp8 (12.6 MB/core vs 16.8 bf16); skip
  leaves as bf16 [N, BG, F]. PSUM evacuation is spread across Act
  (skip pairs), DVE and Pool (v pairs) so no engine exceeds ~85%.
"""

import sys

import numpy as np

for _p in ("/opt/trn_rl_repo",):
    if _p not in sys.path:
        sys.path.insert(0, _p)

import ml_dtypes

import concourse.bacc as bacc
import concourse.bass as bass
import concourse.mybir as mybir
from concourse.bass import MemorySpace
from concourse.tile import TileContext

BF16 = mybir.dt.bfloat16
FP8 = mybir.dt.float8e4
F32 = mybir.dt.float32
AFT = mybir.ActivationFunctionType
DR = mybir.MatmulPerfMode.DoubleRow

B, N, D, H, C = 1024, 128, 256, 4, 64
F = H * C            # 256
NCORES = 8
BG = B // NCORES     # 128 graphs per core
GROUP = 8            # graphs per group (softmax/DMA batch)
NG = BG // GROUP     # 16 groups
ROWS = N + 1         # 129 output rows per graph
WSC = 16.0           # weight pre-scale before fp8 quantization
GL = 3 * N * GROUP   # free bytes per (partition, k-half) per group = 3072

_cached = {}


def _prune_dup_ldweights(nc):
    """Remove PE Ldweights whose weights AP repeats the immediately
    preceding load (the matmul between them keeps using the active
    buffer, so the reload only clogs the single background slot)."""
    removed = 0
    for bb in nc.m.functions[0].blocks:
        insts = bb.instructions
        keep = []
        last_key = None
        fold_into_next_mm = None
        for i in insts:
            if i.opcode == 'Ldweights':
                key = str(i.ins[0])
                if key == last_key:
                    fold_into_next_mm = i
                    removed += 1
                    continue
                last_key = key
            elif i.opcode == 'Matmult':
                if fold_into_next_mm is not None:
                    i.merge_dependencies_from(fold_into_next_mm)
                    fold_into_next_mm = None
            keep.append(i)
        if removed:
            insts[:] = keep
    return removed


def _build_nc():
    nc = bacc.Bacc()

    # inputs: [p, k-half, group, x8|r8|e8 by graph] fp8
    xer_d = nc.dram_tensor("xer", [128, 2, NG, GL], FP8, kind="ExternalInput")
    ws8_d = nc.dram_tensor("ws8", [128, 2, F], FP8, kind="ExternalInput")
    wsr8_d = nc.dram_tensor("wsr8", [128, 2, F], FP8, kind="ExternalInput")
    wv8_d = nc.dram_tensor("wv8", [128, 2, F], FP8, kind="ExternalInput")
    we8_d = nc.dram_tensor("we8", [128, 2, F], FP8, kind="ExternalInput")
    wkq_d = nc.dram_tensor("wkq", [128, 2, BG, H], FP8, kind="ExternalInput")
    weq_d = nc.dram_tensor("weq", [128, 2, BG, H], FP8, kind="ExternalInput")
    ones_d = nc.dram_tensor("ones", [128, 1], BF16, kind="ExternalInput")

    skip_d = nc.dram_tensor("skip", [N, BG, F], BF16, kind="ExternalOutput")
    aggT_d = nc.dram_tensor("aggT", [128, 4, BG], F32, kind="ExternalOutput")
    sums_d = nc.dram_tensor("sums", [1, BG * H], F32, kind="ExternalOutput")

    with TileContext(nc) as tc:
        with (
            tc.tile_pool(name="consts", bufs=1) as consts,
            tc.tile_pool(name="io", bufs=2) as io,
            tc.tile_pool(name="vsb", bufs=GROUP + 2) as v_pool,
            tc.tile_pool(name="skipsb", bufs=2) as skip_pool,
            tc.tile_pool(name="expsb", bufs=3) as exp_pool,
            tc.tile_pool(name="scsum", bufs=2) as scsum_pool,
            tc.tile_pool(name="misc", bufs=4) as misc,
            tc.tile_pool(name="sv_ps", bufs=2, space=MemorySpace.PSUM) as sv_psp,
            tc.tile_pool(name="sc_ps", bufs=2, space=MemorySpace.PSUM) as sc_psp,
            tc.tile_pool(name="agg_ps", bufs=1, space=MemorySpace.PSUM) as agg_psp,
            tc.tile_pool(name="sum_ps", bufs=1, space=MemorySpace.PSUM) as sum_psp,
        ):
            # ---- constants (spread across trigger queues for a fast ramp) ----
            ws8_sb = consts.tile([128, 2, F], FP8, tag="ws8")
            nc.scalar.dma_start(ws8_sb[:, :, :], ws8_d[:, :, :])
            wsr8_sb = consts.tile([128, 2, F], FP8, tag="wsr8")
            nc.scalar.dma_start(wsr8_sb[:, :, :], wsr8_d[:, :, :])
            wv8_sb = consts.tile([128, 2, F], FP8, tag="wv8")
            nc.sync.dma_start(wv8_sb[:, :, :], wv8_d[:, :, :])
            we8_sb = consts.tile([128, 2, F], FP8, tag="we8")
            nc.sync.dma_start(we8_sb[:, :, :], we8_d[:, :, :])
            wkq_sb = consts.tile([128, 2, BG, H], FP8, tag="wkq")
            nc.gpsimd.dma_start(wkq_sb[:, :, :, :], wkq_d[:, :, :, :])
            weq_sb = consts.tile([128, 2, BG, H], FP8, tag="weq")
            nc.gpsimd.dma_start(weq_sb[:, :, :, :], weq_d[:, :, :, :])
            ones_sb = consts.tile([128, 1], BF16, tag="ones")
            nc.sync.dma_start(ones_sb[:, :], ones_d[:, :])

            # persistent PSUM: 4 agg column-regions per graph (half the rows
            # of each are cross-head garbage; host slices the valid halves),
            # plus per-(g,h) exp-sums. start=True only on the first write
            # into each bank.
            agg_ps = agg_psp.tile([128, 4, BG], F32, tag="agg")
            sums_ps = sum_psp.tile([1, BG * H], F32, tag="sums")

            state = {}

            def consume(j, tail=False):
                exp_sb, v_sbs = state.pop(j)
                nc.tensor.matmul(sums_ps[0:1, j * 32:(j + 1) * 32],
                                 ones_sb[:, :], exp_sb[:, :],
                                 start=(j == 0), stop=(j == NG - 1),
                                 skip_group_check=(j > 0))
                for pp in range(GROUP // 2):
                    # v pair tile [128, 2, 256] covers graphs (2pp, 2pp+1)
                    vp = v_sbs[pp]
                    for jj in range(2):
                        gg = pp * 2 + jj
                        g = j * GROUP + gg
                        for half in range(2):
                            # stationary [128n, 128]: v columns for heads
                            # (2*half, 2*half+1)
                            st = vp[:, jj, half * 128:(half + 1) * 128]
                            for hh in range(2):
                                h = half * 2 + hh
                                first = (g == 0 and half == 0 and hh == 0)
                                last = (tail and gg == GROUP - 1
                                        and half == 1 and hh == 1)
                                mm = nc.tensor.matmul(
                                    agg_ps[:, half * 2 + hh, g:g + 1],
                                    st, exp_sb[:, gg * 4 + h:gg * 4 + h + 1],
                                    start=first, stop=last,
                                    skip_group_check=not first)
                                if hh == 1:
                                    mm.ins.ldweights = False

            # ---- main loop over groups of 8 graphs ----
            for grp in range(NG):
                g0 = grp * GROUP
                xer = io.tile([128, 2, GL], FP8, tag="xer")
                if grp == 0:
                    # first tile gates the PE ramp: split across queues
                    hl = GL // 2
                    nc.scalar.dma_start(xer[:, :, 0:hl],
                                        xer_d[:, :, grp, 0:hl])
                    nc.sync.dma_start(xer[:, :, hl:GL],
                                      xer_d[:, :, grp, hl:GL])
                else:
                    # steady state: the scalar HWDGE queue (sync carries the
                    # skip-out stream, gpsimd's SWDGE costs Pool time)
                    nc.scalar.dma_start(xer[:, :, :], xer_d[:, :, grp, :])

                sc_ps = sc_psp.tile([128, 2, GROUP * H], F32, tag="scps")
                skip_t = skip_pool.tile([128, GROUP, F], BF16, tag="skipsb")
                v_sbs = []
                sv_pair = None

                for gg in range(GROUP):
                    g = g0 + gg
                    jj = gg % 2
                    if jj == 0:
                        sv_pair = sv_psp.tile([128, 2, 2 * F], F32, tag="svps")
                    x8 = xer[:, :, gg * N:(gg + 1) * N]
                    r8 = xer[:, :, N * GROUP + gg * N:N * GROUP + (gg + 1) * N]
                    e8 = xer[:, :, 2 * N * GROUP + gg * N:
                             2 * N * GROUP + (gg + 1) * N]
                    sk = sv_pair[:, jj, F:2 * F]
                    vv = sv_pair[:, jj, 0:F]
                    ssl = slice(gg * 4, gg * 4 + 4)

                    # x8 stationary: skip (2 mms), v, scores
                    nc.tensor.matmul(sk, x8, ws8_sb[:, :, :],
                                     start=True, stop=False, perf_mode=DR)
                    nc.tensor.matmul(sk, x8, wsr8_sb[:, :, :],
                                     start=False, stop=False, perf_mode=DR,
                                     skip_group_check=True
                                     ).ins.ldweights = False
                    nc.tensor.matmul(vv, x8, wv8_sb[:, :, :],
                                     start=False, stop=False, perf_mode=DR,
                                     skip_group_check=True
                                     ).ins.ldweights = False
                    nc.tensor.matmul(sc_ps[:, 0, ssl], x8,
                                     wkq_sb[:, :, g, :],
                                     start=(gg == 0), stop=False,
                                     perf_mode=DR,
                                     skip_group_check=(gg > 0)
                                     ).ins.ldweights = False
                    # r8 stationary: skip compensation
                    nc.tensor.matmul(sk, r8, ws8_sb[:, :, :],
                                     start=False, stop=False, perf_mode=DR,
                                     skip_group_check=True)
                    # e8 stationary: v edge part, scores edge part
                    nc.tensor.matmul(vv, e8, we8_sb[:, :, :],
                                     start=False, stop=True, perf_mode=DR,
                                     skip_group_check=True)
                    nc.tensor.matmul(sc_ps[:, 1, ssl], e8,
                                     weq_sb[:, :, g, :],
                                     start=False, stop=(gg == GROUP - 1),
                                     perf_mode=DR,
                                     skip_group_check=True
                                     ).ins.ldweights = False

                    if jj == 1:
                        pp = gg // 2
                        # evacuate the pair: v to DVE/Pool, skip to Act
                        vp = v_pool.tile([128, 2, F], BF16, tag="vsb")
                        nc.vector.tensor_copy(vp[:, :, :], sv_pair[:, :, 0:F])
                        v_sbs.append(vp)
                        nc.scalar.activation(skip_t[:, gg - 1:gg + 1, :],
                                             sv_pair[:, :, F:2 * F], AFT.Copy)
                nc.sync.dma_start(skip_d[:, g0:g0 + GROUP, :], skip_t[:, :, :])

                # scores: two PSUM regions -> DVE add -> Act exp (1/16 fold)
                sc_t = scsum_pool.tile([128, 2, GROUP * H], F32, tag="scsum")
                nc.vector.tensor_copy(sc_t[:, 0, :], sc_ps[:, 0, :])
                nc.vector.tensor_add(sc_t[:, 1, :], sc_ps[:, 1, :],
                                     sc_t[:, 0, :])
                exp_sb = exp_pool.tile([128, GROUP * H], BF16, tag="expsb")
                nc.scalar.activation(exp_sb[:, :], sc_t[:, 1, :], AFT.Exp,
                                     scale=1.0 / WSC)
                state[grp] = (exp_sb, v_sbs)
                if grp >= 1:
                    consume(grp - 1)

            consume(NG - 1, tail=True)

            # ---- ship raw agg / sums; host normalizes & rescales ----
            aggT_sb = misc.tile([128, 4, BG], F32, tag="aggT")
            nc.vector.tensor_copy(aggT_sb[:, :, :], agg_ps[:, :, :])
            nc.sync.dma_start(aggT_d[:, :, :], aggT_sb[:, :, :])
            sums_sb = misc.tile([1, BG * H], F32, tag="sumsb")
            nc.vector.tensor_copy(sums_sb[:, :], sums_ps[:, :])
            nc.sync.dma_start(sums_d[:, :], sums_sb[:, :])

    _prune_dup_ldweights(nc)
    nc.compile()
    return nc


def kernel(**inputs):
    x = np.asarray(inputs["neighbor_node_features"], dtype=np.float32)   # [B, N, D]
    e = np.asarray(inputs["edge_features"], dtype=np.float32)            # [B, N, D]
    cen = np.asarray(inputs["central_node_features"], dtype=np.float32)  # [B, 1, D]
    Wq = np.asarray(inputs["Wq"], dtype=np.float32)
    Wk = np.asarray(inputs["Wk"], dtype=np.float32)
    Wv = np.asarray(inputs["Wv"], dtype=np.float32)
    We = np.asarray(inputs["We"], dtype=np.float32)
    Ws = np.asarray(inputs["Wskip"], dtype=np.float32)
    bq = np.asarray(inputs["bq"], dtype=np.float32)
    # biases are all zeros in this model family (bq folds into q host-side)
    for bn in ("bk", "bv", "bskip"):
        bv = np.asarray(inputs[bn])
        assert np.abs(bv).max() == 0.0, f"nonzero bias {bn} unsupported"

    f8 = ml_dtypes.float8_e4m3fn
    bf = ml_dtypes.bfloat16

    def q8(a):
        return a.astype(f8)

    cT = cen.reshape(B, D).T                                      # [D, B] f32

    # host-side q projection + scaling + fold into per-graph weight columns
    qs = (Wq.T @ cT + bq[:, None]) * (1.0 / np.sqrt(C))           # [F, B]
    qs4 = qs.reshape(H, C, B)
    wkq = np.matmul(Wk.reshape(D, H, C).transpose(1, 0, 2), qs4)  # [H, D, B]
    weq = np.matmul(We.reshape(D, H, C).transpose(1, 0, 2), qs4)
    # -> [D, B, H] then fp8 at 16x and k-halves packed [128, 2, B, H]
    wkq = np.ascontiguousarray(
        q8(WSC * wkq.transpose(1, 2, 0)).reshape(2, 128, B, H)
        .transpose(1, 0, 2, 3))
    weq = np.ascontiguousarray(
        q8(WSC * weq.transpose(1, 2, 0)).reshape(2, 128, B, H)
        .transpose(1, 0, 2, 3))

    def packw(w):
        # [D, F] -> fp8 [128, 2, F] with d = k*128 + p
        return np.ascontiguousarray(w.reshape(2, 128, F).transpose(1, 0, 2))

    ws16 = WSC * Ws
    ws8 = q8(ws16)
    wsr8 = q8(ws16 - ws8.astype(np.float32))
    wv8 = q8(WSC * Wv)
    we8 = q8(WSC * We)
    ones = np.ones((128, 1), dtype=np.float32).astype(bf)

    # x8 / r8 / e8 in [128p, 2k, B, N] then per-core groups
    x8 = q8(x)                                                    # [B, N, D]
    r8 = q8(x - x8.astype(np.float32))
    e8 = q8(e)

    def pack_in(a8):
        # [B, N, D] fp8 -> [128, 2, B, N] with d = k*128 + p
        return a8.transpose(2, 0, 1).reshape(2, 128, B, N).transpose(1, 0, 2, 3)

    x8p, r8p, e8p = pack_in(x8), pack_in(r8), pack_in(e8)
    # combined [128, 2, B(as NG*GROUP), 3, N] -> [128, 2, NG, 3*GROUP*N]
    xer = np.empty((128, 2, B // GROUP, 3, GROUP, N), dtype=f8)
    xer[:, :, :, 0] = x8p.reshape(128, 2, B // GROUP, GROUP, N)
    xer[:, :, :, 1] = r8p.reshape(128, 2, B // GROUP, GROUP, N)
    xer[:, :, :, 2] = e8p.reshape(128, 2, B // GROUP, GROUP, N)
    xer = xer.reshape(128, 2, B // GROUP, 3 * GROUP * N)

    if "nc" not in _cached:
        _cached["nc"] = _build_nc()
    nc = _cached["nc"]

    in_maps = []
    for c in range(NCORES):
        gsl = slice(c * BG, (c + 1) * BG)
        grpsl = slice(c * NG, (c + 1) * NG)
        in_maps.append({
            "xer": np.ascontiguousarray(xer[:, :, grpsl]),
            "wkq": np.ascontiguousarray(wkq[:, :, gsl]),
            "weq": np.ascontiguousarray(weq[:, :, gsl]),
            "ws8": packw(ws8), "wsr8": packw(wsr8),
            "wv8": packw(wv8), "we8": packw(we8),
            "ones": ones,
        })

    from concourse.bass_utils import run_bass_kernel_spmd
    res = run_bass_kernel_spmd(nc, in_maps, core_ids=list(range(NCORES)),
                               **_cached.get("run_kwargs", {}))
    _cached["last_results"] = res

    # central row: skip projection on host (f32), plus normalized agg
    skc = cT.T @ Ws                                               # [B, F]

    inv = np.float32(1.0 / WSC)
    out = np.empty((B, ROWS, F), dtype=np.float32)
    for c, r in enumerate(res.results):
        gsl = slice(c * BG, (c + 1) * BG)
        skip = np.asarray(r["skip"]).astype(np.float32)       # [N, BG, F]
        out[gsl, 1:ROWS, :] = skip.transpose(1, 0, 2) * inv
        aggT = np.asarray(r["aggT"])                          # [128, 4, BG]
        s = np.asarray(r["sums"]).reshape(BG, H)              # [BG, H]
        agg = np.empty((BG, H, C), dtype=np.float32)
        agg[:, 0] = aggT[0:64, 0, :].T
        agg[:, 1] = aggT[64:128, 1, :].T
        agg[:, 2] = aggT[0:64, 2, :].T
        agg[:, 3] = aggT[64:128, 3, :].T
        agg *= inv / s[:, :, None]
        out[gsl, 0, :] = skc[gsl] + agg.reshape(BG, F)
    return out.reshape(B * ROWS, F)
